# revision 22
# baseline (speedup 1.0000x reference)
"""Trainium2 Bass kernel for nn_Attention_9981503996487.

Single-layer attention prefill (B=1, S=4096, H=2048, 16 q-heads, 4 kv-heads,
D=128, RoPE, causal, GQA, empty KV cache at cache_position=0).

Sharding (tensor parallel over heads): core c owns q-heads {2c, 2c+1} and
kv-head c//2.  wq/wk/wv split column-wise, wo row-wise; each core computes a
partial o_proj output (bf16) over its 256 head-channels and the host sums the
8 partials in fp32 (the "all-reduce").

Per-core device program (v2, head-merged softmax):

  A) QKV projection per 512-column sequence chunk: packed [2048, 512]
     weight block (q0|q1|k|v) x hiddenT in quarter-granular SBUF tiles
     (4 DMA descriptors per chunk instead of 16).  RoPE reads the f32
     PSUM directly (cos mul + DMA half-swap of PSUM + sin mul + add);
     V is copied to fp16, transposed via PE to [s, d].
  B) Flash-style causal attention per chunk, BOTH q-heads per kv-tile
     step: two score matmuls write one [128, 1024] PSUM tile, ONE exp
     activation covers both heads (halves ACT instruction overhead);
     fp16 probs tiles hold (kv-pair x 2 heads) [128, 2048]; one PV
     matmul per (head, kv-tile) accumulates a merged [128, 1024] O^T
     PSUM tile; softmax denominators accumulate in fp16 on DVE over
     the merged layout (one add per kv-pair); partition reduction +
     broadcast by ones-matmuls; fast reciprocal; one DVE multiply
     normalizes both heads.
  C) o_proj: out[s_tile, :] accumulated over the two heads' OT slices
     in PSUM; evacuated to bf16 on DVE only (scalar engine stays
     exp-pure) and stored with one [128, 2048] DMA per row tile.

Schedule: A(0) emits only the K/V chains (Q-chains of chunk 0 are
deferred to just before B(0), which runs last) so attention starts
sooner and the drain has PE filler.  B/C order [1..7, 0].  PSUM: shared
2-buf ring (A-chain accumulators, V-transpose, o_proj tiles, den
broadcast) + 2x [128,1024] score tiles + 1 merged [128,1024] O^T tile
= exactly 8 banks.
"""

import math

import numpy as np

S = 4096
HID = 2048
D = 128
NCORES = 8
CH = 512          # query / s-chunk width
NCH = S // CH     # 8 chunks
NT = HID // 128   # 16 contraction tiles
SCALE = 1.0 / math.sqrt(D)


def _build_nc():
    import concourse.bacc as bacc
    import concourse.mybir as mybir
    import concourse.tile as tile

    f32 = mybir.dt.float32
    bf16 = mybir.dt.bfloat16
    fp16 = mybir.dt.float16
    EXP = mybir.ActivationFunctionType.Exp

    nc = bacc.Bacc("TRN2", target_bir_lowering=False, debug=False)

    # hTQ[q*128+p, ci*2048 + b*512 + c] = hidden.T[q*512 + b*128 + p, ci*512 + c]
    hTQ = nc.dram_tensor("hTQ", [512, 4 * S], bf16, kind="ExternalInput")
    # wcatQ[q*128+p, b*512 + oc] = wcat[q*512 + b*128 + p, oc]
    wcatQ = nc.dram_tensor("wcatQ", [512, 2048], bf16, kind="ExternalInput")
    wo2 = nc.dram_tensor("wo2", [256, HID], bf16, kind="ExternalInput")
    cosT = nc.dram_tensor("cosT", [128, S], bf16, kind="ExternalInput")
    sinTs = nc.dram_tensor("sinTs", [128, S], bf16, kind="ExternalInput")
    maskn = nc.dram_tensor("maskn", [128, 128], fp16, kind="ExternalInput")
    ident = nc.dram_tensor("ident", [128, 128], fp16, kind="ExternalInput")
    ones16 = nc.dram_tensor("ones16", [128, 128], fp16, kind="ExternalInput")
    out = nc.dram_tensor("out", [S, HID], bf16, kind="ExternalOutput")

    with tile.TileContext(nc) as tc:
        with tc.tile_pool(name="persist", bufs=1) as persist:
            qt0 = persist.tile([128, S], bf16, name="qt0")
            qt1 = persist.tile([128, S], bf16, name="qt1")
            ktt = persist.tile([128, S], bf16, name="ktt")
            vsb = persist.tile([128, S], fp16, name="vsb")
            cos_sb = persist.tile([128, S], bf16, name="cos_sb")
            sin_sb = persist.tile([128, S], bf16, name="sin_sb")
            wq_sb = [persist.tile([128, 2048], bf16, name=f"wq_sb{q}")
                     for q in range(4)]
            ht0 = [persist.tile([128, 2048], bf16, name=f"ht0_{q}")
                   for q in range(4)]
            wo_sb0 = persist.tile([128, HID], bf16, name="wo_sb0")
            wo_sb1 = persist.tile([128, HID], bf16, name="wo_sb1")
            maskn_sb = persist.tile([128, 128], fp16, name="maskn_sb")
            ones_sb = persist.tile([128, 128], fp16, name="ones_sb")
            id_sb = persist.tile([128, 128], fp16, name="id_sb")

            qts = [qt0, qt1]

            with (
                tc.tile_pool(name="ah", bufs=2) as ah,
                tc.tile_pool(name="ax", bufs=2) as ax,
                tc.tile_pool(name="bp", bufs=4) as bp,
                tc.tile_pool(name="bd", bufs=2) as bd,
                tc.tile_pool(name="br", bufs=2) as brp,
                tc.tile_pool(name="bo", bufs=2) as bo,
                tc.tile_pool(name="co", bufs=3) as co,
                tc.tile_pool(name="psSH", bufs=2, space="PSUM") as psSH,
                tc.tile_pool(name="psST", bufs=2, space="PSUM") as psST,
                tc.tile_pool(name="psOT", bufs=1, space="PSUM") as psOT,
            ):

                def stage_a_loads(ci, startup=False):
                    """DMA the htile quarters + cos/sin for chunk ci.
                    Mid-kernel: 4 batched quarter descriptors on sync.
                    Startup: 16 slice descriptors in contraction order
                    across all 3 DMA queues so the first chain's matmuls
                    start as soon as slice 0 lands."""
                    s0 = ci * CH
                    if ci == 0:
                        tiles = ht0
                    else:
                        tiles = [ah.tile([128, 2048], bf16, name="htq",
                                         tag=f"htq{q}") for q in range(4)]
                    if startup:
                        qs = [nc.sync, nc.scalar, nc.gpsimd]
                        for t in range(NT):
                            q, b = t // 4, t % 4
                            qs[(2 * t + 1) % 3].dma_start(
                                tiles[q][:, b * 512:(b + 1) * 512],
                                hTQ[q * 128:(q + 1) * 128,
                                    ci * 2048 + b * 512:
                                    ci * 2048 + (b + 1) * 512],
                            )
                        qs[0].dma_start(cos_sb[:, s0:s0 + CH],
                                        cosT[:, s0:s0 + CH])
                        qs[2].dma_start(sin_sb[:, s0:s0 + CH],
                                        sinTs[:, s0:s0 + CH])
                    else:
                        for q in range(4):
                            nc.sync.dma_start(
                                tiles[q][:],
                                hTQ[q * 128:(q + 1) * 128,
                                    ci * 2048:(ci + 1) * 2048],
                            )
                        nc.sync.dma_start(cos_sb[:, s0:s0 + CH],
                                          cosT[:, s0:s0 + CH])
                        nc.sync.dma_start(sin_sb[:, s0:s0 + CH],
                                          sinTs[:, s0:s0 + CH])
                    return tiles

                def stage_a_chain(ci, tiles, o):
                    """One QKV projection chain (o in q0,q1,k,v) for chunk
                    ci: 16 accumulating matmuls + RoPE / V-transpose evac."""
                    s0 = ci * CH
                    ps = psSH.tile([128, CH], f32, name="ps_a", tag="ops")
                    for t in range(NT):
                        q, b = t // 4, t % 4
                        wsl = wq_sb[q][:, b * 512 + o * 128:
                                       b * 512 + (o + 1) * 128]
                        nc.tensor.matmul(
                            ps[:], wsl, tiles[q][:, b * 512:(b + 1) * 512],
                            start=(t == 0), stop=(t == NT - 1),
                        )
                    if o < 3:
                        # RoPE: dest = x * cosT + halfswap(x) * signed_sinT
                        # (evac on scalar so the PSUM ring never waits on
                        # the busy vector queue)
                        x_sb = ax.tile([128, CH], bf16, name="x_sb",
                                       tag="evac")
                        nc.scalar.copy(x_sb[:], ps[:])
                        swap = ax.tile([128, CH], bf16, name="swap",
                                       tag="swap")
                        nc.gpsimd.dma_start(swap[0:64, :], x_sb[64:128, :])
                        nc.gpsimd.dma_start(swap[64:128, :], x_sb[0:64, :])
                        t1 = ax.tile([128, CH], bf16, name="t1", tag="t1")
                        nc.vector.tensor_mul(
                            t1[:], x_sb[:], cos_sb[:, s0:s0 + CH])
                        t2 = ax.tile([128, CH], bf16, name="t2", tag="t2")
                        nc.vector.tensor_mul(
                            t2[:], swap[:], sin_sb[:, s0:s0 + CH])
                        dest = [qt0, qt1, ktt][o]
                        nc.vector.tensor_add(
                            dest[:, s0:s0 + CH], t1[:], t2[:])
                    else:
                        # V: copy fp16, transpose [d, s] -> [s, d] blocks
                        x_v = ax.tile([128, CH], fp16, name="x_v", tag="xv")
                        nc.scalar.copy(x_v[:], ps[:])
                        trp = psSH.tile([128, CH], fp16, name="trp",
                                        tag="ops")
                        for b4 in range(4):
                            nc.tensor.transpose(
                                trp[:, b4 * 128:(b4 + 1) * 128],
                                x_v[:, b4 * 128:(b4 + 1) * 128],
                                id_sb[:],
                            )
                        nc.vector.tensor_copy(vsb[:, s0:s0 + CH], trp[:])

                def stage_a(ci):
                    tiles = stage_a_loads(ci)
                    for o in range(4):
                        stage_a_chain(ci, tiles, o)

                def stage_b(ci):
                    """Attention for chunk ci, both heads merged per
                    kv-tile step."""
                    s0 = ci * CH
                    n_kt = 4 * (ci + 1)
                    d0 = 4 * ci  # first diagonal kv-tile index

                    ot_ps = psOT.tile([128, 2 * CH], f32, name="ot_ps",
                                      tag="ot")
                    denA = bd.tile([128, 4 * CH], fp16, name="denA",
                                   tag="denA")
                    denB = bd.tile([128, 4 * CH], fp16, name="denB",
                                   tag="denB")
                    dens = [denA, denB]
                    if ci == 0:
                        nc.gpsimd.memset(denA[:], 0.0)
                        nc.gpsimd.memset(denB[:], 0.0)

                    pair_p = {}

                    def emit(j):
                        diag = j >= d0
                        off = 128 * (j - d0) if diag else 0
                        half = (j % 2) * 2 * CH
                        stp = psST.tile([128, 2 * CH], f32, name="st_ps",
                                        tag="st")
                        for h in range(2):
                            nc.tensor.matmul(
                                stp[:, h * CH + off:(h + 1) * CH],
                                ktt[:, j * 128:(j + 1) * 128],
                                qts[h][:, s0 + off:s0 + CH],
                                start=True, stop=not diag,
                            )
                            if diag:
                                # additive causal mask on the 128-wide
                                # diagonal block: -BIG upper triangle,
                                # accumulated by the PE (id rhs selects
                                # maskn columns); exp flushes to 0.
                                nc.tensor.matmul(
                                    stp[:, h * CH + off:h * CH + off + 128],
                                    maskn_sb[:], id_sb[:],
                                    start=False, stop=True,
                                )
                        jp = j // 2
                        if j % 2 == 0:
                            pair_p[jp] = bp.tile([128, 4 * CH], fp16,
                                                 name="p_sb", tag="p")
                        p = pair_p[jp]
                        if off == 0:
                            nc.scalar.activation(
                                p[:, half:half + 2 * CH], stp[:, 0:2 * CH],
                                EXP, scale=SCALE,
                            )
                        else:
                            for h in range(2):
                                nc.scalar.activation(
                                    p[:, half + h * CH + off:
                                      half + (h + 1) * CH],
                                    stp[:, h * CH + off:(h + 1) * CH],
                                    EXP, scale=SCALE,
                                )

                    def consume(j):
                        off = 128 * (j - d0) if j >= d0 else 0
                        half = (j % 2) * 2 * CH
                        jp = j // 2
                        p = pair_p[jp]
                        for h in range(2):
                            nc.tensor.matmul(
                                ot_ps[:, h * CH + off:(h + 1) * CH],
                                vsb[:, j * 128:(j + 1) * 128],
                                p[:, half + h * CH + off:
                                  half + (h + 1) * CH],
                                start=(j == 0), stop=(j == n_kt - 1),
                            )
                        if j % 2 == 1:
                            den = dens[jp % 2]
                            if j < d0:
                                if ci > 0 and jp < 2:
                                    nc.vector.tensor_copy(den[:], p[:])
                                else:
                                    nc.vector.tensor_add(den[:], den[:],
                                                         p[:])
                            else:
                                # diagonal pair: restricted column ranges
                                r0 = (j - 1) - d0
                                o0 = 128 * r0
                                o1 = 128 * (r0 + 1)
                                for c0, cw in ((o0, 0), (CH + o0, 0),
                                               (2 * CH + o1, 0),
                                               (3 * CH + o1, 0)):
                                    hi = (c0 // CH + 1) * CH
                                    nc.vector.tensor_add(
                                        den[:, c0:hi], den[:, c0:hi],
                                        p[:, c0:hi])
                            del pair_p[jp]

                    skew = 3
                    for j in range(n_kt):
                        emit(j)
                        if j >= skew:
                            consume(j - skew)
                    for j in range(max(0, n_kt - skew), n_kt):
                        consume(j)

                    # finalize: denominators -> reciprocal -> normalize
                    nc.vector.tensor_add(denA[:], denA[:], denB[:])
                    bc0 = psST.tile([128, CH], f32, name="bc0", tag="st")
                    nc.tensor.matmul(bc0[:], ones_sb[:], denA[:, 0:CH],
                                     start=True, stop=False)
                    nc.tensor.matmul(bc0[:], ones_sb[:],
                                     denA[:, 2 * CH:3 * CH],
                                     start=False, stop=True)
                    bc1 = psST.tile([128, CH], f32, name="bc1", tag="st")
                    nc.tensor.matmul(bc1[:], ones_sb[:], denA[:, CH:2 * CH],
                                     start=True, stop=False)
                    nc.tensor.matmul(bc1[:], ones_sb[:],
                                     denA[:, 3 * CH:4 * CH],
                                     start=False, stop=True)
                    rec = brp.tile([128, 2 * CH], f32, name="rec", tag="rec")
                    nc.vector.reciprocal_approx_fast(rec[:, 0:CH], bc0[:])
                    nc.vector.reciprocal_approx_fast(rec[:, CH:2 * CH],
                                                     bc1[:])
                    ot_sb = bo.tile([128, 2 * CH], bf16, name="ot_sb",
                                    tag="ot_sb")
                    nc.vector.tensor_mul(ot_sb[:], ot_ps[:], rec[:])
                    return ot_sb

                def stage_c_group(ci, ot_sb, st_i):
                    row = (ci * 4 + st_i) * 128
                    o_sb = co.tile([128, HID], bf16, name="o_sb",
                                   tag="o_sb")
                    for hc in range(4):
                        ops = psSH.tile([128, 512], f32, name="ops",
                                        tag="ops")
                        nc.tensor.matmul(
                            ops[:],
                            ot_sb[:, st_i * 128:(st_i + 1) * 128],
                            wo_sb0[:, hc * 512:(hc + 1) * 512],
                            start=True, stop=False,
                        )
                        nc.tensor.matmul(
                            ops[:],
                            ot_sb[:, CH + st_i * 128:
                                  CH + (st_i + 1) * 128],
                            wo_sb1[:, hc * 512:(hc + 1) * 512],
                            start=False, stop=True,
                        )
                        dst = o_sb[:, hc * 512:(hc + 1) * 512]
                        if hc % 2 == 0:
                            nc.scalar.copy(dst, ops[:])
                        else:
                            nc.vector.tensor_copy(dst, ops[:])
                    q = nc.sync if st_i % 2 == 0 else nc.gpsimd
                    q.dma_start(out[row:row + 128, :], o_sb[:])

                # ---- emission ----
                # Startup: batched quarter DMAs across 4 queues; chunk-0
                # K/V chains only (Q-chains deferred to just before B(0),
                # which drains last); then A(1) and the small cold loads.
                sq = [nc.sync, nc.scalar, nc.gpsimd]
                # interleave wcat and chunk-0 hidden slices in the order
                # the first K/V chains consume them
                for t in range(NT):
                    q, b = t // 4, t % 4
                    sq[(2 * t) % 3].dma_start(
                        wq_sb[q][:, b * 512:(b + 1) * 512],
                        wcatQ[q * 128:(q + 1) * 128, b * 512:(b + 1) * 512])
                    sq[(2 * t + 1) % 3].dma_start(
                        ht0[q][:, b * 512:(b + 1) * 512],
                        hTQ[q * 128:(q + 1) * 128, b * 512:(b + 1) * 512])
                sq[0].dma_start(cos_sb[:, 0:CH], cosT[:, 0:CH])
                sq[2].dma_start(sin_sb[:, 0:CH], sinTs[:, 0:CH])
                sq[1].dma_start(id_sb[:], ident[:])
                for o in (2, 3):  # k, v chains of chunk 0
                    stage_a_chain(0, ht0, o)
                ht1 = stage_a_loads(1, startup=True)
                nc.gpsimd.dma_start(maskn_sb[:], maskn[:])
                nc.gpsimd.dma_start(ones_sb[:], ones16[:])
                nc.scalar.dma_start(wo_sb0[:], wo2[0:128, :])
                nc.scalar.dma_start(wo_sb1[:], wo2[128:256, :])
                for o in range(4):
                    stage_a_chain(1, ht1, o)

                order = [1, 2, 3, 4, 5, 6, 7, 0]
                for k, ci in enumerate(order):
                    ot_sb = stage_b(ci)
                    # interleave the next QKV chains with this chunk's
                    # o_proj groups so consecutive PSUM-ring slots always
                    # have a fast-clearing reader between A chains
                    if k + 2 < NCH:
                        tiles = stage_a_loads(k + 2)
                        chains = [(k + 2, tiles, o) for o in range(4)]
                    elif ci == 7:
                        # deferred chunk-0 Q chains: PE filler for the
                        # drain; B(0) runs next and needs them.
                        chains = [(0, ht0, 0), (0, ht0, 1)]
                    else:
                        chains = []
                    for st_i in range(4):
                        if st_i < len(chains):
                            stage_a_chain(*chains[st_i])
                        stage_c_group(ci, ot_sb, st_i)

    nc.finalize()
    return nc


def _host_prep(hidden_states, cos, sin, position_ids, wq, wk, wv, wo):
    """Build the 8 per-core input maps."""
    import ml_dtypes
    np_dt = ml_dtypes.bfloat16

    hidden = np.asarray(hidden_states, dtype=np.float32)[0]        # [S, HID]
    hT = np.ascontiguousarray(hidden.T)                            # [HID, S]
    # hTQ[q*128+p, ci*2048 + b*512 + c] = hT[q*512 + b*128 + p, ci*512 + c]
    hTQ = np.ascontiguousarray(
        hT.reshape(4, 4, 128, NCH, CH).transpose(0, 2, 3, 1, 4)
        .reshape(512, 4 * S)
    ).astype(np_dt)

    pos = np.asarray(position_ids)[0].astype(np.int64)             # [S]
    cos_np = np.asarray(cos, dtype=np.float32)[pos]                # [S, 64]
    sin_np = np.asarray(sin, dtype=np.float32)[pos]
    cos_full = np.concatenate([cos_np, cos_np], axis=1)            # [S, 128]
    sin_full = np.concatenate([sin_np, sin_np], axis=1)
    cosT = np.ascontiguousarray(cos_full.T).astype(np_dt)          # [128, S]
    sinTs = np.ascontiguousarray(sin_full.T)
    sinTs[0:64, :] *= -1.0                                         # sign fold
    sinTs = sinTs.astype(np_dt)

    # additive causal mask, pre-transposed for the PE accumulate:
    # maskn[q_local, kv_local] = -30/SCALE where kv > q (upper strict)
    rr = np.arange(128)[:, None]
    cc = np.arange(128)[None, :]
    maskn = np.where(cc > rr, np.float32(-30.0 / SCALE),
                     np.float32(0.0)).astype(np.float16)           # [128, 128]
    ident = np.eye(128, dtype=np.float16)
    ones16 = np.ones((128, 128), dtype=np.float16)

    wq_np = np.asarray(wq, dtype=np.float32)
    wk_np = np.asarray(wk, dtype=np.float32)
    wv_np = np.asarray(wv, dtype=np.float32)
    wo_np = np.asarray(wo, dtype=np.float32)

    in_maps = []
    for c in range(NCORES):
        h0 = 2 * c
        g = c // 2
        wcat = np.concatenate(
            [
                wq_np[:, h0 * D:(h0 + 1) * D],
                wq_np[:, (h0 + 1) * D:(h0 + 2) * D],
                wk_np[:, g * D:(g + 1) * D],
                wv_np[:, g * D:(g + 1) * D],
            ],
            axis=1,
        )                                                          # [HID, 512]
        # wcatQ[q*128+p, b*512 + oc] = wcat[q*512 + b*128 + p, oc]
        wcatQ = np.ascontiguousarray(
            wcat.reshape(4, 4, 128, 512).transpose(0, 2, 1, 3)
            .reshape(512, 2048)
        ).astype(np_dt)
        wo2 = np.ascontiguousarray(
            wo_np[h0 * D:(h0 + 2) * D, :]
        ).astype(np_dt)                                            # [256, HID]
        in_maps.append({
            "hTQ": hTQ,
            "wcatQ": wcatQ,
            "wo2": wo2,
            "cosT": cosT,
            "sinTs": sinTs,
            "maskn": maskn,
            "ident": ident,
            "ones16": ones16,
        })
    return in_maps


_NC_CACHE = [None]


def _run(inputs, trace=False, tmpdir=None):
    from concourse import bass_utils

    in_maps = _host_prep(
        inputs["hidden_states"], inputs["cos"], inputs["sin"],
        inputs["position_ids"], inputs["wq"], inputs["wk"], inputs["wv"],
        inputs["wo"],
    )
    if _NC_CACHE[0] is None:
        _NC_CACHE[0] = _build_nc()
    nc = _NC_CACHE[0]
    res = bass_utils.run_bass_kernel_spmd(
        nc, in_maps, core_ids=list(range(NCORES)), trace=trace, tmpdir=tmpdir,
    )
    acc = res.results[0]["out"].astype(np.float32)
    for c in range(1, NCORES):
        acc = acc + res.results[c]["out"].astype(np.float32)
    return acc.reshape(1, S, HID), res


def kernel(**inputs):
    out, _ = _run(inputs, trace=False)
    return out
